# revision 1
# baseline (speedup 1.0000x reference)
"""CTC loss (reduction='mean', zero_infinity) on 8 Trainium2 NeuronCores.

Strategy (data-parallel over batch, 8 batch elems per core):
  - Stream logits tiles (128 rows = 8b x 16t, 1296 cols) HBM->SBUF once.
  - ACT: e = exp(x - 1) with free-dim accumulation -> per-(b,t) sum S' = S*e^-1.
  - GPSIMD ap_gather: pick the 65 extended-label columns per row.
  - SBUF->SBUF DMA rearranges gathers into a persistent p-hat store laid out
    (8 b-partitions per T/4-quarter, 80 cols per t).
  - CTC forward DP in scaled prob space on DVE (4 tensor_tensor ops/step),
    backward DP on GPSIMD, meeting at TSTAR; periodic renorm anchors the max
    at e^+60 to use the full f32 exponent range.
  - Junction log-sum-exp over lattice states; ln(sum_t ln S) correction and
    constant offsets folded in on-device; host takes mean(nll/target_len).
"""
import numpy as np

import concourse.bass as bass
import concourse.bacc as bacc
import concourse.mybir as mybir
import concourse.tile as tile
from concourse.bass_utils import run_bass_kernel_spmd

f32 = mybir.dt.float32
u16 = mybir.dt.uint16
AF = mybir.ActivationFunctionType
ALU = mybir.AluOpType
AX = mybir.AxisListType

B, T, V, S = 64, 512, 1296, 32
L = 2 * S + 1          # 65
NCORES = 8
BL = B // NCORES       # 8 batch elems per core
W = 80                 # stored cols per t (= gather num_idxs, mult of 16, >= L)
CHUNK = 16             # time steps per memory tile (128 rows / 8 b)
NK = T // CHUNK        # 32 tiles
TSTAR = 330            # forward steps on DVE; backward T-2..TSTAR+1 on GPSIMD
RENORM = 32
KLN = 60.0             # renorm anchor: max -> e^KLN


def _body(nc, tc, lg, idx, msk, msk2, outt, t_total, tstar):
    QT = t_total // 4
    nk = t_total // CHUNK
    nrf = tstar // RENORM
    nrb = (t_total - 2 - tstar) // RENORM
    nr = nrf + nrb
    KF = float(np.exp(np.float32(KLN)))
    SC = float(2.0 ** -64)          # keep Ln inputs inside the ACT domain
    LN2C = float(64 * np.log(2.0))

    with tc.tile_pool(name="const", bufs=1) as cpool, \
         tc.tile_pool(name="lt", bufs=4) as lpool, \
         tc.tile_pool(name="et", bufs=3) as epool, \
         tc.tile_pool(name="gt", bufs=4) as gpool, \
         tc.tile_pool(name="sc", bufs=4) as scpool, \
         tc.tile_pool(name="dp", bufs=3) as dpool:

        WST = L                     # stored cols per t (base partition 0 for all)
        PH = cpool.tile([BL, t_total * WST], f32, tag="PH")
        idx_sb = cpool.tile([128, W // 16], u16, tag="idx")
        nc.sync.dma_start(idx_sb[:], idx)
        M_sb = cpool.tile([BL, L], f32, tag="M")
        nc.sync.dma_start(M_sb[:], msk)
        M2_sb = cpool.tile([BL, L], f32, tag="M2")
        nc.sync.dma_start(M2_sb[:], msk2)
        bm1 = cpool.tile([128, 1], f32, tag="bm1")
        nc.vector.memset(bm1[:], -1.0)
        SAcc = cpool.tile([128, 1], f32, tag="SAcc")
        nc.vector.memset(SAcc[:], 0.0)

        # ---- memory phase: two-ended tile order so fwd and bwd both stream
        order = []
        lo, hi = 0, nk - 1
        while lo <= hi:
            order.append(lo); lo += 1
            if hi >= lo:
                order.append(hi); hi -= 1
        for k in order:
            lt = lpool.tile([128, V], f32, tag="lt")
            src = lg[:, k * CHUNK:(k + 1) * CHUNK, :]
            nc.sync.dma_start(lt[:], src)
            et = epool.tile([128, V], f32, tag="et")
            sc = scpool.tile([128, 1], f32, tag="sc")
            nc.scalar.activation(et[:], lt[:], AF.Exp, bias=bm1[:], accum_out=sc[:])
            gt = gpool.tile([128, W], f32, tag="gt")
            nc.gpsimd.indirect_copy(gt[:], et[:], idx_sb[:], True)
            dst = PH[:, k * CHUNK * WST:(k + 1) * CHUNK * WST]
            nc.sync.dma_start(dst, gt[:, 0:WST])
            lns = scpool.tile([128, 1], f32, tag="lns")
            nc.scalar.activation(lns[:], sc[:], AF.Ln)
            nc.vector.tensor_tensor(SAcc[:], SAcc[:], lns[:], op=ALU.add)

        def ph_at(t):
            return PH[:, t * WST:t * WST + L]

        # ---- forward DP on DVE
        alA = cpool.tile([BL, L + 2], f32, tag="alA")
        alB = cpool.tile([BL, L + 2], f32, tag="alB")
        nc.vector.memset(alA[:], 0.0)
        nc.vector.memset(alB[:], 0.0)
        Cf = cpool.tile([BL, 1], f32, tag="Cf")
        nc.vector.memset(Cf[:], 0.0)
        nc.vector.tensor_scalar_mul(alA[:, 2:4], PH[:, 0:2], KF)
        cur, nxt = alA, alB
        for t in range(1, tstar + 1):
            ph = ph_at(t)
            u = dpool.tile([BL, L], f32, tag="u")
            nc.vector.tensor_tensor(u[:], cur[:, 2:L + 2], cur[:, 1:L + 1], op=ALU.add)
            v = dpool.tile([BL, L], f32, tag="v")
            nc.vector.tensor_tensor(v[:], cur[:, 0:L], M_sb[:], op=ALU.mult)
            nc.vector.tensor_tensor(u[:], u[:], v[:], op=ALU.add)
            nc.vector.tensor_tensor(nxt[:, 2:L + 2], u[:], ph, op=ALU.mult)
            cur, nxt = nxt, cur
            if t % RENORM == 0:
                mx = dpool.tile([BL, 1], f32, tag="mx")
                nc.vector.tensor_reduce(mx[:], cur[:, 2:L + 2], axis=AX.X, op=ALU.max)
                nc.vector.tensor_scalar_max(mx[:], mx[:], 1e-30)
                rc = dpool.tile([BL, 1], f32, tag="rc")
                nc.vector.reciprocal(rc[:], mx[:])
                nc.vector.tensor_scalar(cur[:, 2:L + 2], cur[:, 2:L + 2], rc[:], KF,
                                        op0=ALU.mult, op1=ALU.mult)
                lnm = dpool.tile([BL, 1], f32, tag="lnm")
                nc.scalar.activation(lnm[:], mx[:], AF.Ln, scale=SC)
                nc.vector.tensor_tensor(Cf[:], Cf[:], lnm[:], op=ALU.add)
        alpha = cur  # alpha_{tstar} in cols 2:L+2

        # ---- backward DP on GPSIMD (delta in cols 0:L, zero guards at L:L+2)
        deA = cpool.tile([BL, L + 2], f32, tag="deA")
        deB = cpool.tile([BL, L + 2], f32, tag="deB")
        nc.vector.memset(deA[:], 0.0)
        nc.vector.memset(deB[:], 0.0)
        Cb = cpool.tile([BL, 1], f32, tag="Cb")
        nc.vector.memset(Cb[:], 0.0)
        nc.vector.tensor_scalar_mul(
            deA[:, L - 2:L],
            PH[:, (t_total - 1) * WST + L - 2:(t_total - 1) * WST + L], KF)
        dcur, dnxt = deA, deB
        nsteps = 0
        for t in range(t_total - 2, tstar, -1):
            ph = ph_at(t)
            gu = dpool.tile([BL, L], f32, tag="gu")
            nc.gpsimd.tensor_tensor(gu[:], dcur[:, 0:L], dcur[:, 1:L + 1], op=ALU.add)
            gv = dpool.tile([BL, L], f32, tag="gv")
            nc.gpsimd.tensor_tensor(gv[:], dcur[:, 2:L + 2], M2_sb[:], op=ALU.mult)
            nc.gpsimd.tensor_tensor(gu[:], gu[:], gv[:], op=ALU.add)
            nc.gpsimd.tensor_tensor(dnxt[:, 0:L], gu[:], ph, op=ALU.mult)
            dcur, dnxt = dnxt, dcur
            nsteps += 1
            if nsteps % RENORM == 0:
                mx = dpool.tile([BL, 1], f32, tag="bmx")
                nc.vector.tensor_reduce(mx[:], dcur[:, 0:L], axis=AX.X, op=ALU.max)
                nc.vector.tensor_scalar_max(mx[:], mx[:], 1e-30)
                rc = dpool.tile([BL, 1], f32, tag="brc")
                nc.vector.reciprocal(rc[:], mx[:])
                nc.gpsimd.tensor_scalar(dcur[:, 0:L], dcur[:, 0:L], rc[:], KF,
                                        op0=ALU.mult, op1=ALU.mult)
                lnm = dpool.tile([BL, 1], f32, tag="blnm")
                nc.scalar.activation(lnm[:], mx[:], AF.Ln, scale=SC)
                nc.vector.tensor_tensor(Cb[:], Cb[:], lnm[:], op=ALU.add)

        # gamma_{tstar} = de + sh1(de) + sh2(de)*M2 (no p-hat multiply)
        gam = cpool.tile([BL, L], f32, tag="gam")
        nc.gpsimd.tensor_tensor(gam[:], dcur[:, 0:L], dcur[:, 1:L + 1], op=ALU.add)
        gv = dpool.tile([BL, L], f32, tag="gv2")
        nc.gpsimd.tensor_tensor(gv[:], dcur[:, 2:L + 2], M2_sb[:], op=ALU.mult)
        nc.gpsimd.tensor_tensor(gam[:], gam[:], gv[:], op=ALU.add)

        # ---- junction: lnP = LSE_s(ln alpha + ln gamma) + Cf + Cb - nr*KLN - sum_t lnS
        la = cpool.tile([BL, L], f32, tag="la")
        nc.scalar.activation(la[:], alpha[:, 2:L + 2], AF.Ln, scale=SC)
        lgm = cpool.tile([BL, L], f32, tag="lgm")
        nc.scalar.activation(lgm[:], gam[:], AF.Ln, scale=SC)
        qq = cpool.tile([BL, L], f32, tag="qq")
        nc.vector.tensor_tensor(qq[:], la[:], lgm[:], op=ALU.add)
        mq = cpool.tile([BL, 1], f32, tag="mq")
        nc.vector.tensor_reduce(mq[:], qq[:], axis=AX.X, op=ALU.max)
        nc.vector.tensor_scalar_max(mq[:], mq[:], -1e30)
        dq = cpool.tile([BL, L], f32, tag="dq")
        nc.vector.tensor_scalar_sub(dq[:], qq[:], mq[:])
        eq = cpool.tile([BL, L], f32, tag="eq")
        se = cpool.tile([BL, 1], f32, tag="se")
        nc.scalar.activation(eq[:], dq[:], AF.Exp, accum_out=se[:])
        lnp = cpool.tile([BL, 1], f32, tag="lnp")
        nc.scalar.activation(lnp[:], se[:], AF.Ln)

        # per-b sum of lnS: rearrange (128,1)->(8,16), reduce over free
        sb16 = cpool.tile([BL, 16], f32, tag="sb16")
        nc.sync.dma_start(sb16[:], SAcc[:])
        sb1 = cpool.tile([BL, 1], f32, tag="sb1")
        nc.vector.tensor_reduce(sb1[:], sb16[:], axis=AX.X, op=ALU.add)

        r = cpool.tile([BL, 1], f32, tag="r")
        nc.vector.tensor_tensor(r[:], lnp[:], mq[:], op=ALU.add)
        nc.vector.tensor_tensor(r[:], r[:], Cf[:], op=ALU.add)
        nc.vector.tensor_tensor(r[:], r[:], Cb[:], op=ALU.add)
        nc.vector.tensor_tensor(r[:], r[:], sb1[:], op=ALU.subtract)
        onll = cpool.tile([BL, 1], f32, tag="onll")
        final_bias = float((nr + 2) * KLN - (128 + 64 * nr) * np.log(2.0))
        nc.scalar.activation(onll[:], r[:], AF.Copy, bias=final_bias, scale=-1.0)
        nc.sync.dma_start(outt, onll[:])


KERNEL_VER = 3


def build_bass(t_total=T, tstar=TSTAR):
    nc = bacc.Bacc("TRN2")
    # dummy input whose shape encodes the kernel version: busts stale
    # HLO-hash-keyed executable caches when the BIR changes
    ver = nc.dram_tensor("ver", (1, KERNEL_VER), f32, kind="ExternalInput")
    lg = nc.dram_tensor("logits", (BL, t_total, V), f32, kind="ExternalInput")
    idx = nc.dram_tensor("idx", (128, W // 16), u16, kind="ExternalInput")
    msk = nc.dram_tensor("mask", (BL, L), f32, kind="ExternalInput")
    msk2 = nc.dram_tensor("mask2", (BL, L), f32, kind="ExternalInput")
    outt = nc.dram_tensor("out", (BL, 1), f32, kind="ExternalOutput")
    with tile.TileContext(nc) as tc:
        with tc.tile_pool(name="ver", bufs=1) as vpool:
            vt = vpool.tile([1, KERNEL_VER], f32)
            nc.sync.dma_start(vt[:], ver.ap())
        _body(nc, tc, lg.ap(), idx.ap(), msk.ap(), msk2.ap(), outt.ap(),
              t_total, tstar)
    nc.compile()
    return nc


def host_prep(targets):
    """Per-core gather indices (wrapped), masks M and M2."""
    targets = np.asarray(targets).astype(np.int64)
    ext = np.zeros((B, L), dtype=np.int64)
    ext[:, 1::2] = targets
    pos = np.arange(L)
    ext_m2 = np.full((B, L), -1, dtype=np.int64)
    ext_m2[:, 2:] = ext[:, :-2]
    M = ((pos[None, :] % 2 == 1) & (ext != ext_m2)).astype(np.float32)
    M2 = np.zeros_like(M)
    M2[:, :-2] = M[:, 2:]
    idxs, msks, msk2s = [], [], []
    for c in range(NCORES):
        sl = slice(c * BL, (c + 1) * BL)
        idx_full = np.zeros((BL, W), dtype=np.uint16)
        idx_full[:, :L] = ext[sl].astype(np.uint16)
        idx_w = idx_full.reshape(BL, W // 16, 16).transpose(0, 2, 1).reshape(128, W // 16)
        idxs.append(np.ascontiguousarray(idx_w))
        msks.append(np.ascontiguousarray(M[sl]))
        msk2s.append(np.ascontiguousarray(M2[sl]))
    return idxs, msks, msk2s


_nc_cache = {}


def kernel(logits, targets, input_lengths, target_lengths):
    logits = np.ascontiguousarray(np.asarray(logits), dtype=np.float32)
    targets = np.asarray(targets)
    il = np.asarray(input_lengths)
    tl = np.asarray(target_lengths)
    assert logits.shape == (B, T, V)
    assert int(il.min()) == T and int(il.max()) == T, "kernel specialized to full input_lengths"
    assert int(tl.min()) == S and int(tl.max()) == S, "kernel specialized to full target_lengths"

    if "nc" not in _nc_cache:
        _nc_cache["nc"] = build_bass()
    nc = _nc_cache["nc"]

    idxs, msks, msk2s = host_prep(targets)
    in_maps = []
    for c in range(NCORES):
        sl = slice(c * BL, (c + 1) * BL)
        in_maps.append({
            "ver": np.zeros((1, KERNEL_VER), dtype=np.float32),
            "logits": np.ascontiguousarray(logits[sl]),
            "idx": idxs[c],
            "mask": msks[c],
            "mask2": msk2s[c],
        })
    res = run_bass_kernel_spmd(nc, in_maps, core_ids=list(range(NCORES)))
    nll = np.concatenate([np.asarray(res.results[c]["out"]).reshape(BL)
                          for c in range(NCORES)])
    ok = np.isfinite(nll) & (nll < 1e29)
    nll = np.where(ok, nll, 0.0)
    return np.float32(np.mean(nll / tl.astype(np.float64)))



# revision 21
# speedup vs baseline: 1.2673x; 1.2673x over previous
"""CTC loss (reduction='mean', zero_infinity) on 8 Trainium2 NeuronCores.

Strategy (data-parallel over batch, 8 batch elems per core):
  - Stream logits tiles (128 rows = 8b x 16t, 1296 cols) HBM->SBUF once,
    alternating two HWDGE load queues (SP / ACT).
  - ACT: e = exp(x - 1) with free-dim accumulation -> per-(b,t) sum S' = S/e,
    accumulated into a [128, 32] buffer; a single Ln pass at the end (avoids
    per-tile Exp<->Ln activation-table thrash).
  - GPSIMD ap_gather picks 130 columns per row: the 65 extended-label columns
    (forward order) plus the same 65 reversed (for the backward chain),
    padded to 144. Gathers for a GROUP of tiles stack into one SBUF buffer,
    then ONE SWDGE DMA per group moves them into the p-hat store (grouped
    writes amortize the ~2.5us fixed cost per DMA).
  - P-hat store PH4 [16, 16, 16, 65] = [2*8b, dt, tile, s]: partitions 0-7
    hold ph[b, t, s] for t < 256 (forward chain), partitions 8-15 hold
    ph[b, 511-u, 64-s] at position u (backward chain, time- and lattice-
    reversed so both DP chains read one AP per step). Tiles 0-15 feed only
    the forward half, tiles 16-31 only the backward half.
  - Paired CTC DP: forward alpha (partitions 0-7) and backward delta
    (partitions 8-15, s-reversed so its stencil matches forward's) advance
    in lockstep: 255 steps x 4 DVE tensor ops on [16, 65/67] tiles, renorm
    every 32 steps anchoring the max at e^+60 (renorm maxima batched, one
    Ln pass at the end).
  - Junction at t*=255: gamma from delta (3 ops), DMA-reverse to partitions
    0-7, log-sum-exp over lattice states; ln-sum-S and renorm constants
    folded in on-device; host takes mean(nll/target_len).
"""
import numpy as np

import concourse.bass as bass
import concourse.bacc as bacc
import concourse.mybir as mybir
import concourse.tile as tile
from concourse.bass_utils import run_bass_kernel_spmd

f32 = mybir.dt.float32
u16 = mybir.dt.uint16
AF = mybir.ActivationFunctionType
ALU = mybir.AluOpType
AX = mybir.AxisListType

B, T, V, S = 64, 512, 1296, 32
L = 2 * S + 1          # 65
NCORES = 8
BL = B // NCORES       # 8 batch elems per core
W = 80                 # gather cols per row: 65 used + pad (16-mult)
CHUNK = 16             # time steps per memory tile (128 rows / 8 b)
NK = T // CHUNK        # 32 tiles
TSTAR = (T - 2) // 2   # 255: paired chains, fwd t=1..255, bwd t=510..256
RENORM = 32
NRE = TSTAR // RENORM  # 7 renorm events
KLN = 60.0             # renorm anchor: max -> e^KLN

# tile groups: (is_fwd, [tile indices in gather order]); small leading groups
# so the DP can start early, larger ones after.  fwd tiles ascend, bwd tiles
# descend (natural consumption order of each chain).
GROUPS = [
    (True, [0]), (False, [31]), (True, [1]), (False, [30]),
    (True, [2, 3, 4, 5]), (False, [29, 28, 27, 26]),
    (True, [6, 7, 8, 9]), (False, [25, 24, 23, 22]),
    (True, [10, 11, 12, 13]), (False, [21, 20, 19, 18]),
    (True, [14, 15]), (False, [17, 16]),
]
GMAX = max(len(g[1]) for g in GROUPS)


def _body(nc, tc, lg, idx, msk, outt):
    NR = 2 * NRE                   # 14 per-sample renorm scale factors
    KF = float(np.exp(np.float32(KLN)))
    SC = float(2.0 ** -64)         # keep Ln inputs inside the ACT domain

    with tc.tile_pool(name="const", bufs=1) as cpool, \
         tc.tile_pool(name="lt", bufs=4) as lpool, \
         tc.tile_pool(name="et", bufs=3) as epool, \
         tc.tile_pool(name="gt", bufs=2) as gpool, \
         tc.tile_pool(name="dp", bufs=3) as dpool:

        # [2*8b, dt, tile*W+c] (cols 0:65 of each W-block used); fwd half:
        # t = tile*16+dt; bwd half: u = same, real t = 511-u, s reversed
        # (the reversal is baked into the bwd gather index list)
        PH4 = cpool.tile([2 * BL, CHUNK, (NK // 2) * W], f32, tag="PH4")
        idxF = cpool.tile([128, W // 16], u16, tag="idxF")
        nc.sync.dma_start(idxF[:], idx[:, 0:W // 16])
        idxB = cpool.tile([128, W // 16], u16, tag="idxB")
        nc.sync.dma_start(idxB[:], idx[:, W // 16:2 * (W // 16)])
        M_sb = cpool.tile([2 * BL, L], f32, tag="M")
        nc.sync.dma_start(M_sb[:], msk)
        bm1 = cpool.tile([128, 1], f32, tag="bm1")
        nc.vector.memset(bm1[:], -1.0)
        SAcc = cpool.tile([128, NK], f32, tag="SAcc")

        # ---- memory phase: grouped gathers + one SWDGE PH write per group
        qtoggle = 0
        for is_fwd, tiles in GROUPS:
            G = len(tiles)
            gts = gpool.tile([128, GMAX * W], f32, tag="gts")
            idx_sb = idxF if is_fwd else idxB
            for g, k in enumerate(tiles):
                lt = lpool.tile([128, V], f32, tag="lt")
                srcap = lg[:, k * CHUNK:(k + 1) * CHUNK, :]
                ldq = nc.scalar if (qtoggle < 8 and qtoggle % 2 == 1) else nc.sync
                qtoggle += 1
                ldq.dma_start(lt[:], srcap)
                et = epool.tile([128, V], f32, tag="et")
                nc.scalar.activation(et[:], lt[:], AF.Exp, bias=bm1[:],
                                     accum_out=SAcc[:, k:k + 1])
                nc.gpsimd.indirect_copy(gts[:, g * W:(g + 1) * W], et[:],
                                        idx_sb[:], True)
            if is_fwd:
                a0 = tiles[0]  # ascending run: a-index == tile index
                dst = PH4[0:BL, :, a0 * W:(a0 + G) * W]
            else:
                a0 = 31 - tiles[0]  # descending run: a-index = 31 - tile
                dst = PH4[BL:2 * BL, CHUNK - 1::-1, a0 * W:(a0 + G) * W]
            nc.gpsimd.dma_start(dst, gts[:, 0:G * W])

        # ---- paired DP on DVE: state cols 2:L+2, zero guards at 0:2
        alA = cpool.tile([2 * BL, L + 2], f32, tag="alA")
        alB = cpool.tile([2 * BL, L + 2], f32, tag="alB")
        nc.vector.memset(alA[:], 0.0)
        nc.vector.memset(alB[:], 0.0)
        MXS = cpool.tile([2 * BL, NRE], f32, tag="MXS")
        nc.vector.tensor_scalar_mul(alA[:, 2:4], PH4[:, 0, 0:2], KF)
        cur, nxt = alA, alB
        for t in range(1, TSTAR + 1):
            ph = PH4[:, t % CHUNK, (t // CHUNK) * W:(t // CHUNK) * W + L]
            u = dpool.tile([2 * BL, L], f32, tag="u")
            nc.vector.tensor_tensor(u[:], cur[:, 2:L + 2], cur[:, 1:L + 1], op=ALU.add)
            v = dpool.tile([2 * BL, L], f32, tag="v")
            nc.vector.tensor_tensor(v[:], cur[:, 0:L], M_sb[:], op=ALU.mult)
            nc.vector.tensor_tensor(u[:], u[:], v[:], op=ALU.add)
            nc.vector.tensor_tensor(nxt[:, 2:L + 2], u[:], ph, op=ALU.mult)
            cur, nxt = nxt, cur
            if t % RENORM == 0:
                i = t // RENORM - 1
                mx = MXS[:, i:i + 1]
                nc.vector.tensor_reduce(mx, cur[:, 2:L + 2], axis=AX.X, op=ALU.max)
                nc.vector.tensor_scalar_max(mx, mx, 1e-30)
                rc = dpool.tile([2 * BL, 1], f32, tag="rc")
                nc.vector.reciprocal(rc[:], mx)
                nc.vector.tensor_scalar(cur[:, 2:L + 2], cur[:, 2:L + 2], rc[:], KF,
                                        op0=ALU.mult, op1=ALU.mult)
        # fwd half of cur = alpha_255; bwd half = delta_256 (s-reversed)

        # gamma* = de + sh1(de) + sh2(de)*M2~ (no p-hat multiply); fwd half junk
        gam = cpool.tile([2 * BL, L], f32, tag="gam")
        nc.vector.tensor_tensor(gam[:], cur[:, 2:L + 2], cur[:, 1:L + 1], op=ALU.add)
        gv = dpool.tile([2 * BL, L], f32, tag="gv")
        nc.vector.tensor_tensor(gv[:], cur[:, 0:L], M_sb[:], op=ALU.mult)
        nc.vector.tensor_tensor(gam[:], gam[:], gv[:], op=ALU.add)

        # renorm scale logs, batched: Cacc[p] = sum_i ln(mx_i * SC)
        lnmx = cpool.tile([2 * BL, NRE], f32, tag="lnmx")
        nc.scalar.activation(lnmx[:], MXS[:], AF.Ln, scale=SC)
        Cacc = cpool.tile([2 * BL, 1], f32, tag="Cacc")
        nc.vector.tensor_reduce(Cacc[:], lnmx[:], axis=AX.X, op=ALU.add)

        # per-b sum of ln S': Ln over [128, NK], reduce, regroup (8,16), reduce
        lns32 = cpool.tile([128, NK], f32, tag="lns32")
        nc.scalar.activation(lns32[:], SAcc[:], AF.Ln)
        red = cpool.tile([128, 1], f32, tag="red")
        nc.vector.tensor_reduce(red[:], lns32[:], axis=AX.X, op=ALU.add)
        sb16 = cpool.tile([BL, 16], f32, tag="sb16")
        nc.sync.dma_start(sb16[:], red[:])
        sb1 = cpool.tile([BL, 1], f32, tag="sb1")
        nc.vector.tensor_reduce(sb1[:], sb16[:], axis=AX.X, op=ALU.add)

        # ---- output: the final 65-wide LSE junction runs on the host in
        # float64 (the ACT Ln LUT saturates near 1.2e-20, corrupting a
        # device-side LSE).  Ship alpha, gamma (still s-reversed), Cacc, sb1.
        ob = cpool.tile([2 * BL, L + 2], f32, tag="ob")
        nc.vector.tensor_scalar_mul(ob[:, 0:1], Cacc[:], 1.0)
        nc.vector.tensor_scalar_mul(ob[0:BL, 1:2], sb1[:], 1.0)
        nc.vector.tensor_scalar_mul(ob[:, 2:L + 2], gam[:], 1.0)
        nc.vector.tensor_scalar_mul(ob[0:BL, 2:L + 2], cur[0:BL, 2:L + 2], 1.0)
        nc.sync.dma_start(outt, ob[:])


KERNEL_VER = 6


def build_bass():
    nc = bacc.Bacc("TRN2")
    # dummy input whose shape encodes the kernel version: busts stale
    # HLO-hash-keyed executable caches when the BIR changes
    ver = nc.dram_tensor("ver", (1, KERNEL_VER), f32, kind="ExternalInput")
    lg = nc.dram_tensor("logits", (BL, T, V), f32, kind="ExternalInput")
    idx = nc.dram_tensor("idx", (128, 2 * (W // 16)), u16, kind="ExternalInput")
    msk = nc.dram_tensor("mask", (2 * BL, L), f32, kind="ExternalInput")
    outt = nc.dram_tensor("out", (2 * BL, L + 2), f32, kind="ExternalOutput")
    with tile.TileContext(nc) as tc:
        with tc.tile_pool(name="ver", bufs=1) as vpool:
            vt = vpool.tile([1, KERNEL_VER], f32)
            nc.sync.dma_start(vt[:], ver.ap())
        _body(nc, tc, lg.ap(), idx.ap(), msk.ap(), outt.ap())
    nc.compile()
    return nc


def host_prep(targets):
    """Per-core gather indices (wrapped) and the paired skip mask."""
    targets = np.asarray(targets).astype(np.int64)
    ext = np.zeros((B, L), dtype=np.int64)
    ext[:, 1::2] = targets
    pos = np.arange(L)
    ext_m2 = np.full((B, L), -1, dtype=np.int64)
    ext_m2[:, 2:] = ext[:, :-2]
    M = ((pos[None, :] % 2 == 1) & (ext != ext_m2)).astype(np.float32)
    M2 = np.zeros_like(M)
    M2[:, :-2] = M[:, 2:]
    idxs, msks = [], []
    for c in range(NCORES):
        sl = slice(c * BL, (c + 1) * BL)
        def wrap(lst):
            full = np.zeros((BL, W), dtype=np.uint16)
            full[:, :L] = lst.astype(np.uint16)
            return full.reshape(BL, W // 16, 16).transpose(0, 2, 1).reshape(128, W // 16)
        idx_w = np.concatenate([wrap(ext[sl]), wrap(ext[sl, ::-1])], axis=1)
        idxs.append(np.ascontiguousarray(idx_w))
        # paired mask: fwd M on partitions 0-7, s-reversed M2 on 8-15
        mp = np.concatenate([M[sl], M2[sl, ::-1]], axis=0).astype(np.float32)
        msks.append(np.ascontiguousarray(mp))
    return idxs, msks


_nc_cache = {}


def kernel(logits, targets, input_lengths, target_lengths):
    logits = np.ascontiguousarray(np.asarray(logits), dtype=np.float32)
    targets = np.asarray(targets)
    il = np.asarray(input_lengths)
    tl = np.asarray(target_lengths)
    assert logits.shape == (B, T, V)
    assert int(il.min()) == T and int(il.max()) == T, "kernel specialized to full input_lengths"
    assert int(tl.min()) == S and int(tl.max()) == S, "kernel specialized to full target_lengths"

    if "nc" not in _nc_cache:
        _nc_cache["nc"] = build_bass()
    nc = _nc_cache["nc"]

    idxs, msks = host_prep(targets)
    in_maps = []
    for c in range(NCORES):
        sl = slice(c * BL, (c + 1) * BL)
        in_maps.append({
            "ver": np.zeros((1, KERNEL_VER), dtype=np.float32),
            "logits": np.ascontiguousarray(logits[sl]),
            "idx": idxs[c],
            "mask": msks[c],
        })
    res = run_bass_kernel_spmd(nc, in_maps, core_ids=list(range(NCORES)))
    # host epilogue (float64): per-sample 65-wide LSE junction + constants
    NR = 2 * NRE
    cst = 16.0 * KLN - (64.0 * NR) * np.log(2.0)
    nlls = []
    for c in range(NCORES):
        o = np.asarray(res.results[c]["out"]).astype(np.float64)  # (16, 67)
        CaccF, CaccB = o[0:BL, 0], o[BL:2 * BL, 0]
        sb1 = o[0:BL, 1]
        alpha = o[0:BL, 2:L + 2]
        gamma = o[BL:2 * BL, 2:L + 2][:, ::-1]  # un-reverse s
        with np.errstate(divide="ignore"):
            qq = np.log(alpha) + np.log(gamma)
        mq = np.max(qq, axis=1)
        safe = np.isfinite(mq)
        lse = np.where(
            safe,
            mq + np.log(np.exp(qq - np.where(safe, mq, 0.0)[:, None]).sum(axis=1)),
            -np.inf)
        nlls.append(-lse - CaccF - CaccB + sb1 + cst)
    nll = np.concatenate(nlls)
    ok = np.isfinite(nll) & (nll < 1e29)
    nll = np.where(ok, nll, 0.0)
    return np.float32(np.mean(nll / tl.astype(np.float64)))
